# revision 30
# baseline (speedup 1.0000x reference)
"""Trainium2 Bass kernel for nn_AttModel (B=8, S=96, D=768, R=24, RSEQ=8, TAG=3).

Data-parallel over batch: core i handles sample i. v2 design:

  1. Host pre-converts proj_W / rel_W(permuted) / pair-selectors to bf16;
     W DMA is ~21us instead of 42us.
  2. Refine scan in score space with LAGGED max (reduce_max runs off the
     critical chain; exp(s_t - max(s_{t-1})) <= e^30, safe in f32) and the
     softmax normalizer folded into the tiny [8,8] G-scale instead of a
     [8,96] w-scale.
  3. H projections in bf16, two layouts:
     - feature-major hh/ht [128, 96] for k-tiles 0..KD-1 (DVE-direct build)
     - natural-layout combined tiles [128, KP*128] for k-tiles KD..17:
       partitions 0..95 = Ht_nat(j), 96..127 = Hh_nat rows 32b..32b+31,
       hh rows matmul'd into PSUM partitions 96..127 via tile_position.
  4. Main loop per group g (4 i's x 96 j's = 384 pairs), KD=8:
     - k 8..17: pair-matmul  P = combined[:, kslice].T @ selR[g%8]  on PE
       (static 0/1 selector staged from host), then relu PSUM->SBUF bf16
       copies on ACT; pair psums rotate across 6 banks from 3 pools
     - k 0..7: 4x per-i tensor_scalar add+relu on DVE
     - 18 accumulating main matmuls rwr[k].T @ V[k] -> out psum [72, 384]
     Pairs/DVE-builds of group g are interleaved one-per-slot between the
     main matmuls of group g-1, so PE/ACT/DVE all stay ~90% busy.
Output per core: [72, 9216] with channel c = tag*24 + rel (rel_W pre-permuted
on host), reshaped on host to [3, 24, 96, 96].
"""
import sys

sys.path.insert(0, "/opt/trn_rl_repo")

import numpy as np

S, D, H3 = 96, 768, 2304
R, RSEQ, TAG, C = 24, 8, 3, 72
B = 8
KT = D // 128           # 6 d-chunks per half of proj_W
MT = H3 // 128          # 18 feature tiles
KD = 8                  # k-tiles 0..KD-1 built DVE-direct
KP = MT - KD            # k-tiles 6..17 built via pair-matmul
PECOLS = KP * 128       # 1536 features in combined tiles
IGRP = 4
NG = S // IGRP          # 24 groups
NFREE = IGRP * S        # 384
NB = S // 32            # 3 combined blocks (32 i's each)
GPB = 32 // IGRP        # 8 groups per block
ACT_COPY_K = set(range(KD, MT))   # all pair-copies on ACT
SCALE = 1.0 / float(np.sqrt(np.float32(D)))


def build_nc(repeat: int = 1, debug: bool = False):
    import concourse.bass as bass
    from concourse import bacc, mybir
    import concourse.tile as tile
    from concourse.masks import make_identity

    f32 = mybir.dt.float32
    bf16 = mybir.dt.bfloat16
    AF = mybir.ActivationFunctionType
    ALU = mybir.AluOpType
    AX = mybir.AxisListType

    nc = bacc.Bacc()
    enc = nc.dram_tensor("enc", [S, D], f32, kind="ExternalInput")
    arel = nc.dram_tensor("arel", [RSEQ, D], f32, kind="ExternalInput")
    pw16 = nc.dram_tensor("pw16", [2 * D, H3], bf16, kind="ExternalInput")
    relw16 = nc.dram_tensor("relw16", [H3, C], bf16, kind="ExternalInput")
    selr_d = nc.dram_tensor("selr", [128, GPB * NFREE], bf16,
                            kind="ExternalInput")
    pbfm_d = nc.dram_tensor("pbfm", [128, MT], f32, kind="ExternalInput")
    pbnat_d = nc.dram_tensor("pbnat", [1, H3], bf16, kind="ExternalInput")
    out = nc.dram_tensor("out", [C, S * S], f32, kind="ExternalOutput")
    if debug:
        dbg_comb = nc.dram_tensor("dbg_comb", [128, PECOLS], f32,
                                  kind="ExternalOutput")
        dbg_v = nc.dram_tensor("dbg_v", [128, 3 * NFREE], f32,
                               kind="ExternalOutput")
        dbg_scan = nc.dram_tensor("dbg_scan", [RSEQ, S + 8], f32,
                                  kind="ExternalOutput")
        dbg_fm = nc.dram_tensor("dbg_fm", [128, 2 * S], f32,
                                kind="ExternalOutput")

    with tile.TileContext(nc) as tc:
        with (
            tc.tile_pool(name="persist", bufs=1) as pp,
            tc.tile_pool(name="work", bufs=2) as wp,
            tc.tile_pool(name="vpool", bufs=6) as vp,
            tc.tile_pool(name="scanp", bufs=2) as sp,
            tc.tile_pool(name="psmall", bufs=2, space="PSUM") as pss,
            tc.tile_pool(name="psone", bufs=1, space="PSUM") as ps1,
            tc.tile_pool(name="pspair", bufs=3, space="PSUM") as psq,
            tc.tile_pool(name="psout", bufs=2, space="PSUM") as pso,
        ):
            # ---------------- loads ----------------
            ident = pp.tile([128, 128], f32, tag="ident")
            make_identity(nc, ident[:])

            enc_nat = pp.tile([S, D], f32, tag="enc_nat")
            nc.sync.dma_start(enc_nat[:], enc[:])
            a_nat = pp.tile([RSEQ, D], f32, tag="a_nat")
            nc.sync.dma_start(a_nat[:], arel[:])
            selr = pp.tile([128, GPB * NFREE], bf16, tag="selr")
            nc.sync.dma_start(selr[:], selr_d[:])
            pbfm = pp.tile([128, MT], f32, tag="pbfm")
            nc.sync.dma_start(pbfm[:], pbfm_d[:])
            pbnat = pp.tile([1, H3], bf16, tag="pbnat")
            nc.sync.dma_start(pbnat[:], pbnat_d[:])
            rwrb = pp.tile([128, MT * C], bf16, tag="rwrb")
            nc.sync.dma_start(
                rwrb[:].rearrange("p (k c) -> p k c", k=MT),
                relw16.rearrange("(k p) c -> p k c", p=128))
            rwr = [rwrb[:, k * C:(k + 1) * C] for k in range(MT)]
            wb = pp.tile([128, 2 * KT * H3], bf16, tag="wb")
            nc.sync.dma_start(
                wb[:].rearrange("p (n m) -> p n m", n=2 * KT),
                pw16.rearrange("(n p) m -> p n m", p=128))
            w16 = [wb[:, d * H3:(d + 1) * H3] for d in range(2 * KT)]

            ones16 = pp.tile([1, S], bf16, tag="ones16")
            nc.vector.memset(ones16[:], 1.0)

            # ---------------- transposes / scan prep (f32) ----------------
            bT = []
            for k in range(KT):
                if k % 2 == 0:
                    ps = psq.tile([128, S], f32, tag="pairps")
                else:
                    ps = pss.tile([128, S], f32, tag="tps")
                nc.tensor.transpose(
                    ps[:], enc_nat[:, k * 128:(k + 1) * 128], ident[:S, :S])
                t = pp.tile([128, S], f32, tag=f"bT{k}")
                nc.scalar.copy(t[:], ps[:])
                bT.append(t)
            at_scl = []
            for k in range(KT):
                if k % 2 == 0:
                    ps = psq.tile([128, RSEQ], f32, tag="pairps")
                else:
                    ps = pss.tile([128, RSEQ], f32, tag="tps")
                nc.tensor.transpose(
                    ps[:], a_nat[:, k * 128:(k + 1) * 128],
                    ident[:RSEQ, :RSEQ])
                ts = pp.tile([128, RSEQ], f32, tag=f"ats{k}")
                nc.scalar.mul(ts[:], ps[:], SCALE)
                at_scl.append(ts)

            gps = pss.tile([RSEQ, RSEQ], f32, tag="tps")
            for k in range(KT):
                nc.tensor.matmul(gps[:], at_scl[k][:], at_scl[k][:],
                                 start=(k == 0), stop=(k == KT - 1))
            # gps = scale^2 * A@A.T; fold one 1/scale back in
            g_sb = pp.tile([RSEQ, RSEQ], f32, tag="g_sb")
            nc.vector.tensor_scalar_mul(g_sb[:], gps[:], 1.0 / SCALE)

            s_ps = ps1.tile([RSEQ, S], f32, tag="s_ps")
            for k in range(KT):
                nc.tensor.matmul(s_ps[:], at_scl[k][:], bT[k][:],
                                 start=(k == 0), stop=False,
                                 skip_group_check=True)

            # ---------------- scan (lagged max, G-folded normalizer) -------
            wsum = pp.tile([RSEQ, S], f32, tag="wsum")
            nc.vector.memset(wsum[:], 0.0)
            negmax = sp.tile([RSEQ, 1], f32, tag="negmax")
            nc.vector.reduce_max(negmax[:], s_ps[:], axis=AX.X, negate=True)
            for t in range(R):
                u = sp.tile([RSEQ, S], f32, tag="u")
                rs = sp.tile([RSEQ, 1], f32, tag="rs")
                nc.scalar.activation(u[:], s_ps[:], AF.Exp, bias=negmax[:],
                                     scale=1.0, accum_out=rs[:])
                rinv = sp.tile([RSEQ, 1], f32, tag="rinv")
                nc.vector.reciprocal(rinv[:], rs[:])
                if t < R - 1:
                    # critical chain: gsc = G * rinv, then the matmul.
                    gsc = sp.tile([RSEQ, RSEQ], f32, tag="gsc")
                    nc.vector.tensor_scalar_mul(gsc[:], g_sb[:], rinv[:])
                    # lagged max: read s_t BEFORE the matmul updates it; the
                    # next exp sees s_{t+1} - max(s_t) <= ~30, safe in f32.
                    negmax = sp.tile([RSEQ, 1], f32, tag="negmax")
                    nc.vector.reduce_max(negmax[:], s_ps[:], axis=AX.X,
                                         negate=True)
                    nc.tensor.matmul(s_ps[:], gsc[:], u[:],
                                     start=False, stop=(t == R - 2),
                                     skip_group_check=True)
                # wsum += u * rinv (off critical chain)
                nc.vector.scalar_tensor_tensor(
                    wsum[:], u[:], rinv[:], wsum[:],
                    op0=ALU.mult, op1=ALU.add)

            # ---------------- b update + bf16 ----------------
            bT16 = []
            for k in range(KT):
                ps = pss.tile([128, S], f32, tag="tps")
                nc.tensor.matmul(ps[:], a_nat[:, k * 128:(k + 1) * 128],
                                 wsum[:], start=True, stop=True)
                nc.vector.tensor_tensor(bT[k][:], bT[k][:], ps[:], op=ALU.add)
                t16 = pp.tile([128, S], bf16, tag=f"bT16_{k}")
                nc.vector.tensor_scalar_mul(t16[:], bT[k][:], 1.0)
                bT16.append(t16)

            # ---------------- H projections ----------------
            # combined tiles: [0:96] = Ht_nat, [96:128] = Hh_nat rows 32b..
            # Both parts land in ONE psum tile per (block, chunk): the hh
            # rows are matmul'd straight into psum partitions 96..127 via
            # the auto-derived tile_position (M=32 at base 96).
            comb = []
            for b in range(NB):
                t = pp.tile([128, PECOLS], bf16, tag=f"comb{b}")
                comb.append(t)
            def comb_unit(b, cidx):
                lo = cidx * 512
                n = min(512, PECOLS - lo)
                ps = pss.tile([128, 512], f32, tag="tps")
                for d in range(KT):
                    nc.tensor.matmul(
                        ps[0:S, :n], bT16[d][:],
                        wb[:, (KT + d) * H3 + KD * 128 + lo:(KT + d) * H3 + KD * 128 + lo + n],
                        start=(d == 0), stop=False,
                        skip_group_check=True)
                for d in range(KT):
                    nc.tensor.matmul(
                        ps[S:128, :n], bT16[d][:, 32 * b:32 * b + 32],
                        wb[:, d * H3 + KD * 128 + lo:d * H3 + KD * 128 + lo + n],
                        start=(d == 0), stop=False,
                        skip_group_check=True, tile_position=(0, 96))
                nc.tensor.matmul(ps[S:128, :n],
                                 ones16[:, 32 * b:32 * b + 32],
                                 pbnat[:, KD * 128 + lo:KD * 128 + lo + n],
                                 start=False, stop=True,
                                 skip_group_check=True,
                                 tile_position=(0, 96))
                nc.scalar.activation(comb[b][:, lo:lo + n], ps[:, :n],
                                     AF.Identity, scale=1.0)

            for cidx in range(3):
                comb_unit(0, cidx)

            # feature-major hh (f32 + pb) / ht (bf16) for k-tiles 0..5
            hh_fm, ht_fm = [], []
            for k in range(KD):
                msl = slice(k * 128, (k + 1) * 128)
                ps = pss.tile([128, S], f32, tag="tps")
                for d in range(KT):
                    nc.tensor.matmul(ps[:], wb[:, d * H3 + msl.start:d * H3 + msl.stop], bT16[d][:],
                                     start=(d == 0), stop=(d == KT - 1))
                th = pp.tile([128, S], f32, tag=f"hhfm{k}")
                nc.scalar.activation(th[:], ps[:], AF.Identity,
                                     bias=pbfm[:, k:k + 1], scale=1.0)
                hh_fm.append(th)
                ps2 = pss.tile([128, S], f32, tag="tps")
                for d in range(KT):
                    nc.tensor.matmul(ps2[:], wb[:, (KT + d) * H3 + msl.start:(KT + d) * H3 + msl.stop], bT16[d][:],
                                     start=(d == 0), stop=(d == KT - 1))
                tt = pp.tile([128, S], bf16, tag=f"htfm{k}")
                nc.vector.tensor_scalar_mul(tt[:], ps2[:], 1.0)
                ht_fm.append(tt)

            # ---------------- main loop (software-pipelined) ---------------
            def build(g):
                b, gb = g // GPB, g % GPB
                vt = {}
                for k in range(KD, MT):
                    r6 = (k - KD) % 6
                    if r6 < 3:
                        pq = psq.tile([128, NFREE], f32, tag="pairps")
                    elif r6 < 5:
                        pq = pss.tile([128, NFREE], f32, tag="tps")
                    else:
                        pq = ps1.tile([128, NFREE], f32, tag="s_ps")
                    nc.tensor.matmul(
                        pq[:], comb[b][:, (k - KD) * 128:(k - KD + 1) * 128],
                        selr[:, gb * NFREE:(gb + 1) * NFREE],
                        start=True, stop=True)
                    v = vp.tile([128, NFREE], bf16, tag=f"v{k}")
                    if k in ACT_COPY_K:
                        nc.scalar.activation(v[:], pq[:], AF.Relu, scale=1.0)
                    else:
                        nc.vector.tensor_scalar_max(v[:], pq[:], 0.0)
                    vt[k] = v
                for k in range(KD):
                    v = vp.tile([128, NFREE], bf16, tag=f"v{k}")
                    for ii in range(IGRP):
                        i = g * IGRP + ii
                        nc.vector.tensor_scalar(
                            v[:, ii * S:(ii + 1) * S], ht_fm[k][:],
                            hh_fm[k][:, i:i + 1], 0.0,
                            op0=ALU.add, op1=ALU.max)
                    vt[k] = v
                return vt

            def mains(g, vt, pair_emitter=None):
                ops = pso.tile([C, NFREE], f32, tag="ops")
                for j, k in enumerate(range(MT)):
                    if pair_emitter is not None:
                        pair_emitter(j)
                    nc.tensor.matmul(ops[:], rwr[k], vt[k][:],
                                     start=(j == 0), stop=(j == MT - 1))
                ostg = wp.tile([C, NFREE], f32, tag="ostg")
                if g % 2 == 0:
                    nc.scalar.activation(ostg[:], ops[:], AF.Identity,
                                         scale=1.0)
                else:
                    nc.vector.tensor_scalar_mul(ostg[:], ops[:], 1.0)
                nc.sync.dma_start(out[:, g * NFREE:(g + 1) * NFREE], ostg[:])

            def build_paired(g, vt_out):
                """Returns a per-slot emitter interleaving pairs(g) and DVE
                builds(g) into the PE/DVE streams between mains(g-1)."""
                b, gb = g // GPB, g % GPB

                def emit(j):
                    k = KD + j
                    if k < MT:
                        r6 = (k - KD) % 6
                        if r6 < 3:
                            pq = psq.tile([128, NFREE], f32, tag="pairps")
                        elif r6 < 5:
                            pq = pss.tile([128, NFREE], f32, tag="tps")
                        else:
                            pq = ps1.tile([128, NFREE], f32, tag="s_ps")
                        nc.tensor.matmul(
                            pq[:],
                            comb[b][:, (k - KD) * 128:(k - KD + 1) * 128],
                            selr[:, gb * NFREE:(gb + 1) * NFREE],
                            start=True, stop=True)
                        v = vp.tile([128, NFREE], bf16, tag=f"v{k}")
                        if k in ACT_COPY_K:
                            nc.scalar.activation(v[:], pq[:], AF.Relu,
                                                 scale=1.0)
                        else:
                            nc.vector.tensor_scalar_max(v[:], pq[:], 0.0)
                        vt_out[k] = v
                    elif k - MT < KD:
                        kd = k - MT
                        v = vp.tile([128, NFREE], bf16, tag=f"v{kd}")
                        for ii in range(IGRP):
                            i = g * IGRP + ii
                            nc.vector.tensor_scalar(
                                v[:, ii * S:(ii + 1) * S], ht_fm[kd][:],
                                hh_fm[kd][:, i:i + 1], 0.0,
                                op0=ALU.add, op1=ALU.max)
                        vt_out[kd] = v
                return emit

            prev = build(0)
            if debug:
                stg = wp.tile([128, 3 * NFREE], f32, tag="dbgv")
                nc.vector.tensor_scalar_mul(stg[:, :NFREE], prev[0][:], 1.0)
                nc.vector.tensor_scalar_mul(
                    stg[:, NFREE:2 * NFREE], prev[6][:], 1.0)
                nc.vector.tensor_scalar_mul(
                    stg[:, 2 * NFREE:], prev[17][:], 1.0)
                nc.sync.dma_start(dbg_v[:], stg[:])
                stg2 = wp.tile([128, PECOLS], f32, tag="dbgc")
                nc.vector.tensor_scalar_mul(stg2[:], comb[0][:], 1.0)
                nc.sync.dma_start(dbg_comb[:], stg2[:])
                stg3 = wp.tile([RSEQ, S + 8], f32, tag="dbgs")
                nc.vector.tensor_scalar_mul(stg3[:, :S], wsum[:], 1.0)
                nc.vector.tensor_scalar_mul(stg3[:, S:], g_sb[:], 1.0)
                nc.sync.dma_start(dbg_scan[:], stg3[:])
                stg4 = wp.tile([128, 2 * S], f32, tag="dbgf")
                nc.vector.tensor_scalar_mul(stg4[:, :S], hh_fm[0][:], 1.0)
                nc.vector.tensor_scalar_mul(stg4[:, S:], ht_fm[0][:], 1.0)
                nc.sync.dma_start(dbg_fm[:], stg4[:])
            for g in range(1, NG):
                if g in (1, 4, 7):
                    comb_unit(1, (1, 4, 7).index(g))
                elif g in (9, 12, 15):
                    comb_unit(2, (9, 12, 15).index(g))
                cur = {}
                emitter = build_paired(g, cur)
                mains(g - 1, prev, pair_emitter=emitter)
                prev = cur
            mains(NG - 1, prev)

    nc.finalize()
    return nc


_CACHED_NC = None


def _host_consts():
    import ml_dtypes
    bf = ml_dtypes.bfloat16
    sel = np.zeros((128, GPB * NFREE), np.float32)
    for gb in range(GPB):
        base = gb * NFREE
        for ii in range(IGRP):
            sel[np.arange(S), base + ii * S + np.arange(S)] = 1.0
            sel[S + gb * IGRP + ii, base + ii * S:base + (ii + 1) * S] = 1.0
    return sel.astype(bf)


def _prep_in_maps(encoded_text, rel_types_encoded, proj_W, proj_b, rel_W):
    import ml_dtypes
    bf = ml_dtypes.bfloat16
    relw_perm = np.ascontiguousarray(
        rel_W.reshape(H3, R, TAG).transpose(0, 2, 1).reshape(H3, C)
    ).astype(bf)
    pw16 = np.ascontiguousarray(proj_W).astype(bf)
    selr = _host_consts()
    pb32 = np.asarray(proj_b, dtype=np.float32)
    pbfm = np.ascontiguousarray(pb32.reshape(MT, 128).T)  # [128, MT]
    pbnat = pb32.reshape(1, H3).astype(bf)
    in_maps = []
    for i in range(B):
        in_maps.append({
            "enc": np.ascontiguousarray(encoded_text[i], dtype=np.float32),
            "arel": np.ascontiguousarray(
                rel_types_encoded[i], dtype=np.float32),
            "pw16": pw16,
            "relw16": relw_perm,
            "selr": selr,
            "pbfm": pbfm,
            "pbnat": pbnat,
        })
    return in_maps


def _assemble(results, rel_b):
    outs = []
    for i in range(B):
        o = results[i]["out"].reshape(TAG, R, S, S)
        outs.append(o)
    full = np.stack(outs, axis=0).astype(np.float32)
    if np.any(rel_b):
        relb_perm = np.asarray(rel_b, dtype=np.float32).reshape(R, TAG).T
        full = full + relb_perm[None, :, :, None, None]
    return full


def kernel(encoded_text, rel_types_encoded, proj_W, proj_b, rel_W, rel_b):
    global _CACHED_NC
    from concourse.bass_utils import run_bass_kernel_spmd

    if _CACHED_NC is None:
        _CACHED_NC = build_nc()
    in_maps = _prep_in_maps(
        encoded_text, rel_types_encoded, proj_W, proj_b, rel_W)
    res = run_bass_kernel_spmd(_CACHED_NC, in_maps, list(range(B)))
    return _assemble(res.results, rel_b)


# revision 31
# speedup vs baseline: 1.0409x; 1.0409x over previous
"""Trainium2 Bass kernel for nn_AttModel (B=8, S=96, D=768, R=24, RSEQ=8, TAG=3).

Data-parallel over batch: core i handles sample i. v2 design:

  1. Host pre-converts proj_W / rel_W(permuted) / pair-selectors to bf16;
     W DMA is ~21us instead of 42us.
  2. Refine scan in score space with LAGGED max (reduce_max runs off the
     critical chain; exp(s_t - max(s_{t-1})) <= e^30, safe in f32) and the
     softmax normalizer folded into the tiny [8,8] G-scale instead of a
     [8,96] w-scale.
  3. H projections in bf16, two layouts:
     - feature-major hh/ht [128, 96] for k-tiles 0..KD-1 (DVE-direct build)
     - natural-layout combined tiles [128, KP*128] for k-tiles KD..17:
       partitions 0..95 = Ht_nat(j), 96..127 = Hh_nat rows 32b..32b+31,
       hh rows matmul'd into PSUM partitions 96..127 via tile_position.
  4. Main loop per group g (4 i's x 96 j's = 384 pairs), KD=8:
     - k 8..17: pair-matmul  P = combined[:, kslice].T @ selR[g%8]  on PE
       (static 0/1 selector staged from host), then relu PSUM->SBUF bf16
       copies on ACT; pair psums rotate across 6 banks from 3 pools
     - k 0..7: 4x per-i tensor_scalar add+relu on DVE
     - 18 accumulating main matmuls rwr[k].T @ V[k] -> out psum [72, 384]
     Pairs/DVE-builds of group g are interleaved one-per-slot between the
     main matmuls of group g-1, so PE/ACT/DVE all stay ~90% busy.
Output per core: [72, 9216] with channel c = tag*24 + rel (rel_W pre-permuted
on host), reshaped on host to [3, 24, 96, 96].
"""
import sys

sys.path.insert(0, "/opt/trn_rl_repo")

import numpy as np

S, D, H3 = 96, 768, 2304
R, RSEQ, TAG, C = 24, 8, 3, 72
B = 8
KT = D // 128           # 6 d-chunks per half of proj_W
MT = H3 // 128          # 18 feature tiles
KD = 8                  # k-tiles 0..KD-1 built DVE-direct
KP = MT - KD            # k-tiles 6..17 built via pair-matmul
PECOLS = KP * 128       # 1536 features in combined tiles
IGRP = 4
NG = S // IGRP          # 24 groups
NFREE = IGRP * S        # 384
NB = S // 32            # 3 combined blocks (32 i's each)
GPB = 32 // IGRP        # 8 groups per block
ACT_COPY_K = set(range(KD, MT))   # all pair-copies on ACT
SCALE = 1.0 / float(np.sqrt(np.float32(D)))


def build_nc(repeat: int = 1, debug: bool = False):
    import concourse.bass as bass
    from concourse import bacc, mybir
    import concourse.tile as tile
    from concourse.masks import make_identity

    f32 = mybir.dt.float32
    bf16 = mybir.dt.bfloat16
    AF = mybir.ActivationFunctionType
    ALU = mybir.AluOpType
    AX = mybir.AxisListType

    nc = bacc.Bacc()
    enc = nc.dram_tensor("enc", [S, D], f32, kind="ExternalInput")
    arel = nc.dram_tensor("arel", [RSEQ, D], f32, kind="ExternalInput")
    pw16 = nc.dram_tensor("pw16", [2 * D, H3], bf16, kind="ExternalInput")
    relw16 = nc.dram_tensor("relw16", [H3, C], bf16, kind="ExternalInput")
    selr_d = nc.dram_tensor("selr", [128, GPB * NFREE], bf16,
                            kind="ExternalInput")
    pbfm_d = nc.dram_tensor("pbfm", [128, MT], f32, kind="ExternalInput")
    pbnat_d = nc.dram_tensor("pbnat", [1, H3], bf16, kind="ExternalInput")
    out = nc.dram_tensor("out", [C, S * S], f32, kind="ExternalOutput")
    if debug:
        dbg_comb = nc.dram_tensor("dbg_comb", [128, PECOLS], f32,
                                  kind="ExternalOutput")
        dbg_v = nc.dram_tensor("dbg_v", [128, 3 * NFREE], f32,
                               kind="ExternalOutput")
        dbg_scan = nc.dram_tensor("dbg_scan", [RSEQ, S + 8], f32,
                                  kind="ExternalOutput")
        dbg_fm = nc.dram_tensor("dbg_fm", [128, 2 * S], f32,
                                kind="ExternalOutput")

    with tile.TileContext(nc) as tc:
        with (
            tc.tile_pool(name="persist", bufs=1) as pp,
            tc.tile_pool(name="work", bufs=2) as wp,
            tc.tile_pool(name="vpool", bufs=4) as vp,
            tc.tile_pool(name="scanp", bufs=2) as sp,
            tc.tile_pool(name="psmall", bufs=2, space="PSUM") as pss,
            tc.tile_pool(name="psone", bufs=1, space="PSUM") as ps1,
            tc.tile_pool(name="pspair", bufs=3, space="PSUM") as psq,
            tc.tile_pool(name="psout", bufs=2, space="PSUM") as pso,
        ):
            # ---------------- loads ----------------
            ident = pp.tile([128, 128], f32, tag="ident")
            make_identity(nc, ident[:])

            enc_nat = pp.tile([S, D], f32, tag="enc_nat")
            nc.sync.dma_start(enc_nat[:], enc[:])
            a_nat = pp.tile([RSEQ, D], f32, tag="a_nat")
            nc.sync.dma_start(a_nat[:], arel[:])
            selr = pp.tile([128, GPB * NFREE], bf16, tag="selr")
            nc.sync.dma_start(selr[:], selr_d[:])
            pbfm = pp.tile([128, MT], f32, tag="pbfm")
            nc.sync.dma_start(pbfm[:], pbfm_d[:])
            pbnat = pp.tile([1, H3], bf16, tag="pbnat")
            nc.sync.dma_start(pbnat[:], pbnat_d[:])
            rwrb = pp.tile([128, MT * C], bf16, tag="rwrb")
            nc.sync.dma_start(
                rwrb[:].rearrange("p (k c) -> p k c", k=MT),
                relw16.rearrange("(k p) c -> p k c", p=128))
            rwr = [rwrb[:, k * C:(k + 1) * C] for k in range(MT)]
            wb = pp.tile([128, 2 * KT * H3], bf16, tag="wb")
            nc.sync.dma_start(
                wb[:].rearrange("p (n m) -> p n m", n=2 * KT),
                pw16.rearrange("(n p) m -> p n m", p=128))
            w16 = [wb[:, d * H3:(d + 1) * H3] for d in range(2 * KT)]

            ones16 = pp.tile([1, S], bf16, tag="ones16")
            nc.vector.memset(ones16[:], 1.0)

            # ---------------- transposes / scan prep (f32) ----------------
            bT = []
            for k in range(KT):
                if k % 2 == 0:
                    ps = psq.tile([128, S], f32, tag="pairps")
                else:
                    ps = pss.tile([128, S], f32, tag="tps")
                nc.tensor.transpose(
                    ps[:], enc_nat[:, k * 128:(k + 1) * 128], ident[:S, :S])
                t = pp.tile([128, S], f32, tag=f"bT{k}")
                nc.scalar.copy(t[:], ps[:])
                bT.append(t)
            at_scl = []
            for k in range(KT):
                if k % 2 == 0:
                    ps = psq.tile([128, RSEQ], f32, tag="pairps")
                else:
                    ps = pss.tile([128, RSEQ], f32, tag="tps")
                nc.tensor.transpose(
                    ps[:], a_nat[:, k * 128:(k + 1) * 128],
                    ident[:RSEQ, :RSEQ])
                ts = pp.tile([128, RSEQ], f32, tag=f"ats{k}")
                nc.scalar.mul(ts[:], ps[:], SCALE)
                at_scl.append(ts)

            gps = pss.tile([RSEQ, RSEQ], f32, tag="tps")
            for k in range(KT):
                nc.tensor.matmul(gps[:], at_scl[k][:], at_scl[k][:],
                                 start=(k == 0), stop=(k == KT - 1))
            # gps = scale^2 * A@A.T; fold one 1/scale back in
            g_sb = pp.tile([RSEQ, RSEQ], f32, tag="g_sb")
            nc.vector.tensor_scalar_mul(g_sb[:], gps[:], 1.0 / SCALE)

            s_ps = ps1.tile([RSEQ, S], f32, tag="s_ps")
            for k in range(KT):
                nc.tensor.matmul(s_ps[:], at_scl[k][:], bT[k][:],
                                 start=(k == 0), stop=False,
                                 skip_group_check=True)

            # ---------------- scan (lagged max, G-folded normalizer) -------
            wsum = pp.tile([RSEQ, S], f32, tag="wsum")
            nc.vector.memset(wsum[:], 0.0)
            negmax = sp.tile([RSEQ, 1], f32, tag="negmax")
            nc.vector.reduce_max(negmax[:], s_ps[:], axis=AX.X, negate=True)
            for t in range(R):
                u = sp.tile([RSEQ, S], f32, tag="u")
                rs = sp.tile([RSEQ, 1], f32, tag="rs")
                nc.scalar.activation(u[:], s_ps[:], AF.Exp, bias=negmax[:],
                                     scale=1.0, accum_out=rs[:])
                rinv = sp.tile([RSEQ, 1], f32, tag="rinv")
                nc.vector.reciprocal(rinv[:], rs[:])
                if t < R - 1:
                    # critical chain: gsc = G * rinv, then the matmul.
                    gsc = sp.tile([RSEQ, RSEQ], f32, tag="gsc")
                    nc.vector.tensor_scalar_mul(gsc[:], g_sb[:], rinv[:])
                    # lagged max: read s_t BEFORE the matmul updates it; the
                    # next exp sees s_{t+1} - max(s_t) <= ~30, safe in f32.
                    negmax = sp.tile([RSEQ, 1], f32, tag="negmax")
                    nc.vector.reduce_max(negmax[:], s_ps[:], axis=AX.X,
                                         negate=True)
                    nc.tensor.matmul(s_ps[:], gsc[:], u[:],
                                     start=False, stop=(t == R - 2),
                                     skip_group_check=True)
                # wsum += u * rinv (off critical chain)
                nc.vector.scalar_tensor_tensor(
                    wsum[:], u[:], rinv[:], wsum[:],
                    op0=ALU.mult, op1=ALU.add)

            # ---------------- b update + bf16 ----------------
            bT16 = []
            for k in range(KT):
                ps = pss.tile([128, S], f32, tag="tps")
                nc.tensor.matmul(ps[:], a_nat[:, k * 128:(k + 1) * 128],
                                 wsum[:], start=True, stop=True)
                nc.vector.tensor_tensor(bT[k][:], bT[k][:], ps[:], op=ALU.add)
                t16 = pp.tile([128, S], bf16, tag=f"bT16_{k}")
                nc.vector.tensor_scalar_mul(t16[:], bT[k][:], 1.0)
                bT16.append(t16)

            # ---------------- H projections ----------------
            # combined tiles: [0:96] = Ht_nat, [96:128] = Hh_nat rows 32b..
            # Both parts land in ONE psum tile per (block, chunk): the hh
            # rows are matmul'd straight into psum partitions 96..127 via
            # the auto-derived tile_position (M=32 at base 96).
            comb = []
            for b in range(NB):
                t = pp.tile([128, PECOLS], bf16, tag=f"comb{b}")
                comb.append(t)
            def comb_unit(b, cidx):
                lo = cidx * 512
                n = min(512, PECOLS - lo)
                ps = pss.tile([128, 512], f32, tag="tps")
                for d in range(KT):
                    nc.tensor.matmul(
                        ps[0:S, :n], bT16[d][:],
                        wb[:, (KT + d) * H3 + KD * 128 + lo:(KT + d) * H3 + KD * 128 + lo + n],
                        start=(d == 0), stop=False,
                        skip_group_check=True)
                for d in range(KT):
                    nc.tensor.matmul(
                        ps[S:128, :n], bT16[d][:, 32 * b:32 * b + 32],
                        wb[:, d * H3 + KD * 128 + lo:d * H3 + KD * 128 + lo + n],
                        start=(d == 0), stop=False,
                        skip_group_check=True, tile_position=(0, 96))
                nc.tensor.matmul(ps[S:128, :n],
                                 ones16[:, 32 * b:32 * b + 32],
                                 pbnat[:, KD * 128 + lo:KD * 128 + lo + n],
                                 start=False, stop=True,
                                 skip_group_check=True,
                                 tile_position=(0, 96))
                nc.scalar.activation(comb[b][:, lo:lo + n], ps[:, :n],
                                     AF.Identity, scale=1.0)

            for cidx in range(3):
                comb_unit(0, cidx)

            # feature-major hh (f32 + pb) / ht (bf16) for k-tiles 0..5
            hh_fm, ht_fm = [], []
            for k in range(KD):
                msl = slice(k * 128, (k + 1) * 128)
                ps = pss.tile([128, S], f32, tag="tps")
                for d in range(KT):
                    nc.tensor.matmul(ps[:], wb[:, d * H3 + msl.start:d * H3 + msl.stop], bT16[d][:],
                                     start=(d == 0), stop=(d == KT - 1))
                th = pp.tile([128, S], f32, tag=f"hhfm{k}")
                nc.scalar.activation(th[:], ps[:], AF.Identity,
                                     bias=pbfm[:, k:k + 1], scale=1.0)
                hh_fm.append(th)
                ps2 = pss.tile([128, S], f32, tag="tps")
                for d in range(KT):
                    nc.tensor.matmul(ps2[:], wb[:, (KT + d) * H3 + msl.start:(KT + d) * H3 + msl.stop], bT16[d][:],
                                     start=(d == 0), stop=(d == KT - 1))
                tt = pp.tile([128, S], bf16, tag=f"htfm{k}")
                nc.vector.tensor_scalar_mul(tt[:], ps2[:], 1.0)
                ht_fm.append(tt)

            # ---------------- main loop (software-pipelined) ---------------
            def build(g):
                b, gb = g // GPB, g % GPB
                vt = {}
                for k in range(KD, MT):
                    r6 = (k - KD) % 6
                    if r6 < 3:
                        pq = psq.tile([128, NFREE], f32, tag="pairps")
                    elif r6 < 5:
                        pq = pss.tile([128, NFREE], f32, tag="tps")
                    else:
                        pq = ps1.tile([128, NFREE], f32, tag="s_ps")
                    nc.tensor.matmul(
                        pq[:], comb[b][:, (k - KD) * 128:(k - KD + 1) * 128],
                        selr[:, gb * NFREE:(gb + 1) * NFREE],
                        start=True, stop=True)
                    v = vp.tile([128, NFREE], bf16, tag=f"v{k}")
                    if k in ACT_COPY_K:
                        nc.scalar.activation(v[:], pq[:], AF.Relu, scale=1.0)
                    else:
                        nc.vector.tensor_scalar_max(v[:], pq[:], 0.0)
                    vt[k] = v
                for k in range(KD):
                    v = vp.tile([128, NFREE], bf16, tag=f"v{k}")
                    for ii in range(IGRP):
                        i = g * IGRP + ii
                        nc.vector.tensor_scalar(
                            v[:, ii * S:(ii + 1) * S], ht_fm[k][:],
                            hh_fm[k][:, i:i + 1], 0.0,
                            op0=ALU.add, op1=ALU.max)
                    vt[k] = v
                return vt

            def mains(g, vt, pair_emitter=None):
                ops = pso.tile([C, NFREE], f32, tag="ops")
                for j, k in enumerate(range(MT)):
                    if pair_emitter is not None:
                        pair_emitter(j)
                    nc.tensor.matmul(ops[:], rwr[k], vt[k][:],
                                     start=(j == 0), stop=(j == MT - 1))
                ostg = wp.tile([C, NFREE], f32, tag="ostg")
                if g % 2 == 0:
                    nc.scalar.activation(ostg[:], ops[:], AF.Identity,
                                         scale=1.0)
                else:
                    nc.vector.tensor_scalar_mul(ostg[:], ops[:], 1.0)
                nc.sync.dma_start(out[:, g * NFREE:(g + 1) * NFREE], ostg[:])

            def build_paired(g, vt_out):
                """Returns a per-slot emitter interleaving pairs(g) and DVE
                builds(g) into the PE/DVE streams between mains(g-1)."""
                b, gb = g // GPB, g % GPB

                def emit(j):
                    k = KD + j
                    if k < MT:
                        r6 = (k - KD) % 6
                        if r6 < 3:
                            pq = psq.tile([128, NFREE], f32, tag="pairps")
                        elif r6 < 5:
                            pq = pss.tile([128, NFREE], f32, tag="tps")
                        else:
                            pq = ps1.tile([128, NFREE], f32, tag="s_ps")
                        nc.tensor.matmul(
                            pq[:],
                            comb[b][:, (k - KD) * 128:(k - KD + 1) * 128],
                            selr[:, gb * NFREE:(gb + 1) * NFREE],
                            start=True, stop=True)
                        v = vp.tile([128, NFREE], bf16, tag=f"v{k}")
                        if k in ACT_COPY_K:
                            nc.scalar.activation(v[:], pq[:], AF.Relu,
                                                 scale=1.0)
                        else:
                            nc.vector.tensor_scalar_max(v[:], pq[:], 0.0)
                        vt_out[k] = v
                    elif k - MT < KD:
                        kd = k - MT
                        v = vp.tile([128, NFREE], bf16, tag=f"v{kd}")
                        for ii in range(IGRP):
                            i = g * IGRP + ii
                            nc.vector.tensor_scalar(
                                v[:, ii * S:(ii + 1) * S], ht_fm[kd][:],
                                hh_fm[kd][:, i:i + 1], 0.0,
                                op0=ALU.add, op1=ALU.max)
                        vt_out[kd] = v
                return emit

            prev = build(0)
            if debug:
                stg = wp.tile([128, 3 * NFREE], f32, tag="dbgv")
                nc.vector.tensor_scalar_mul(stg[:, :NFREE], prev[0][:], 1.0)
                nc.vector.tensor_scalar_mul(
                    stg[:, NFREE:2 * NFREE], prev[6][:], 1.0)
                nc.vector.tensor_scalar_mul(
                    stg[:, 2 * NFREE:], prev[17][:], 1.0)
                nc.sync.dma_start(dbg_v[:], stg[:])
                stg2 = wp.tile([128, PECOLS], f32, tag="dbgc")
                nc.vector.tensor_scalar_mul(stg2[:], comb[0][:], 1.0)
                nc.sync.dma_start(dbg_comb[:], stg2[:])
                stg3 = wp.tile([RSEQ, S + 8], f32, tag="dbgs")
                nc.vector.tensor_scalar_mul(stg3[:, :S], wsum[:], 1.0)
                nc.vector.tensor_scalar_mul(stg3[:, S:], g_sb[:], 1.0)
                nc.sync.dma_start(dbg_scan[:], stg3[:])
                stg4 = wp.tile([128, 2 * S], f32, tag="dbgf")
                nc.vector.tensor_scalar_mul(stg4[:, :S], hh_fm[0][:], 1.0)
                nc.vector.tensor_scalar_mul(stg4[:, S:], ht_fm[0][:], 1.0)
                nc.sync.dma_start(dbg_fm[:], stg4[:])
            for g in range(1, NG):
                if g in (2, 4, 6) or g in (10, 12, 14):
                    comb_unit(1 if g < 8 else 2, (g % 8) // 2 - 1)
                cur = {}
                emitter = build_paired(g, cur)
                mains(g - 1, prev, pair_emitter=emitter)
                prev = cur
            mains(NG - 1, prev)

    nc.finalize()
    return nc


_CACHED_NC = None


def _host_consts():
    import ml_dtypes
    bf = ml_dtypes.bfloat16
    sel = np.zeros((128, GPB * NFREE), np.float32)
    for gb in range(GPB):
        base = gb * NFREE
        for ii in range(IGRP):
            sel[np.arange(S), base + ii * S + np.arange(S)] = 1.0
            sel[S + gb * IGRP + ii, base + ii * S:base + (ii + 1) * S] = 1.0
    return sel.astype(bf)


def _prep_in_maps(encoded_text, rel_types_encoded, proj_W, proj_b, rel_W):
    import ml_dtypes
    bf = ml_dtypes.bfloat16
    relw_perm = np.ascontiguousarray(
        rel_W.reshape(H3, R, TAG).transpose(0, 2, 1).reshape(H3, C)
    ).astype(bf)
    pw16 = np.ascontiguousarray(proj_W).astype(bf)
    selr = _host_consts()
    pb32 = np.asarray(proj_b, dtype=np.float32)
    pbfm = np.ascontiguousarray(pb32.reshape(MT, 128).T)  # [128, MT]
    pbnat = pb32.reshape(1, H3).astype(bf)
    in_maps = []
    for i in range(B):
        in_maps.append({
            "enc": np.ascontiguousarray(encoded_text[i], dtype=np.float32),
            "arel": np.ascontiguousarray(
                rel_types_encoded[i], dtype=np.float32),
            "pw16": pw16,
            "relw16": relw_perm,
            "selr": selr,
            "pbfm": pbfm,
            "pbnat": pbnat,
        })
    return in_maps


def _assemble(results, rel_b):
    outs = []
    for i in range(B):
        o = results[i]["out"].reshape(TAG, R, S, S)
        outs.append(o)
    full = np.stack(outs, axis=0).astype(np.float32)
    if np.any(rel_b):
        relb_perm = np.asarray(rel_b, dtype=np.float32).reshape(R, TAG).T
        full = full + relb_perm[None, :, :, None, None]
    return full


def kernel(encoded_text, rel_types_encoded, proj_W, proj_b, rel_W, rel_b):
    global _CACHED_NC
    from concourse.bass_utils import run_bass_kernel_spmd

    if _CACHED_NC is None:
        _CACHED_NC = build_nc()
    in_maps = _prep_in_maps(
        encoded_text, rel_types_encoded, proj_W, proj_b, rel_W)
    res = run_bass_kernel_spmd(_CACHED_NC, in_maps, list(range(B)))
    return _assemble(res.results, rel_b)
